# revision 2
# baseline (speedup 1.0000x reference)
"""AlphaEntmaxRouter (alpha=1.5) Trainium2 kernel, v2.

Full inputs -> full output. Data-parallel over 8 NeuronCores (token dim
sharded 4096/core), weights replicated.

Per core:
  - x [4096, 2048] fp32 streamed via plain HWDGE DMAs on the SP queue (full
    rate; no SWDGE cast): six 512-token blocks, then four 256-token
    sub-blocks so the post-stream pipeline tail is short.
  - PE transposes each fp32 x chunk in transpose mode (fp32, 2 cyc/row)
    into PSUM; the ACT PSUM->SBUF copies downcast to fp16 in flight.
  - W-stationary fp16 matmul (1 cyc/row) accumulates logits^T
    [64, NG*128] in fp32 PSUM over the 16 k-tiles; bias + the 0.5 entmax
    scale are folded into the transposed weights / epilogue; PE
    re-transposes logits into s = 0.5*(x@W.T+b) laid out
    [128 part, 32 group, 64 expert].
  - entmax-1.5 tau from 4 Newton updates + final eval (5 evals) on
    f(tau) = sum_e relu(s_e - tau)^2 - 1 starting at tau0 = max(s)-1
    (monotone convergence from below; ~2e-3 rel err on this data vs the
    reference's 25-step bisection, well under the 2e-2 gate).
    Every eval op stays on DVE (fused scalar_tensor_tensor forms at the
    2x "2x_2p" rate where eligible) so each unit is a back-to-back
    throughput-bound chain with no cross-engine stalls.  Units are emitted
    interleaved into the stream right after their source block so the
    engine's in-order queue overlaps solver work with later blocks.
  - p = relu(s-tau)^2 normalized by its sum, DMA'd out per unit on the ACT
    HWDGE queue, deferred to the next block's emission point so the queue
    never parks on an unfinished unit.
  - A post-schedule pass (_legalize_waits) splits multi-wait instructions
    for this walrus build.
"""

import numpy as np

N_TOKENS = 32768
D = 2048
E = 64
N_CORES = 8
TOK_PER_CORE = N_TOKENS // N_CORES  # 4096
KT = D // 128  # 16 k-tiles
N_EVALS = 5  # 4 Newton updates + final eval

_BUILT = None


def _build():
    global _BUILT
    if _BUILT is not None:
        return _BUILT

    from contextlib import ExitStack

    import concourse.bass as bass
    import concourse.tile as tile
    from concourse import mybir
    from concourse.masks import make_identity

    f32 = mybir.dt.float32
    OP = mybir.AluOpType
    AF = mybir.ActivationFunctionType
    AX = mybir.AxisListType

    GROUPS = TOK_PER_CORE // 128  # 32
    # blocks as group ranges: 6 fat blocks then 4 small ones for a short tail
    BLOCKS = [(4 * i, 4 * i + 4) for i in range(8)]
    # every block is followed by its own solver unit (1:1), so solver work
    # trails the stream by exactly one block pipeline
    UNITS_AFTER_BLOCK = {i: [rng] for i, rng in enumerate(BLOCKS)}

    nc = bass.Bass("TRN2", debug=False)
    x = nc.dram_tensor("x", [TOK_PER_CORE, D], f32, kind="ExternalInput").ap()
    W = nc.dram_tensor("W", [E, D], f32, kind="ExternalInput").ap()
    b = nc.dram_tensor("b", [E, 1], f32, kind="ExternalInput").ap()
    out = nc.dram_tensor("out", [TOK_PER_CORE, E], f32, kind="ExternalOutput").ap()

    # token t = p*32 + g
    x_v = x.rearrange("(p g) d -> p g d", p=128)
    out_v = out.rearrange("(p g) e -> p g e", p=128)

    def bcast(ap2d, n):
        """[P, G] AP -> [P, G, n] stride-0 broadcast AP."""
        return bass.AP(tensor=ap2d.tensor, offset=ap2d.offset, ap=[*ap2d.ap, [0, n]])

    with tile.TileContext(nc) as tc, ExitStack() as ctx:
        singles = ctx.enter_context(tc.tile_pool(name="singles", bufs=1))
        xin_pool = ctx.enter_context(tc.tile_pool(name="xin", bufs=3))
        xs_pool = ctx.enter_context(tc.tile_pool(name="xs", bufs=4))
        xt_pool = ctx.enter_context(tc.tile_pool(name="xt", bufs=3))
        lg_pool = ctx.enter_context(tc.tile_pool(name="lg", bufs=2))
        big_pool = ctx.enter_context(tc.tile_pool(name="big", bufs=2))
        sm_pool = ctx.enter_context(tc.tile_pool(name="sm", bufs=2))
        tp_psum = ctx.enter_context(tc.tile_pool(name="tp_ps", bufs=2, space="PSUM"))
        lg_psum = ctx.enter_context(tc.tile_pool(name="lg_ps", bufs=2, space="PSUM"))
        s_psum = ctx.enter_context(tc.tile_pool(name="s_ps", bufs=2, space="PSUM"))

        # ---- constants / weight prep -------------------------------------
        f16 = mybir.dt.float16
        ident = singles.tile([128, 128], f32)
        make_identity(nc, ident)

        w_nat = singles.tile([64, D], f32)
        nc.sync.dma_start(out=w_nat, in_=W)
        # Wait-absorber: transpose-mode matmul's LDW struct only fits one
        # sync wait; soak the DMA-completion wait with a tiny standalone
        # bf16 ldweights so later PE instructions inherit it in order.
        nc.tensor.ldweights(w_nat[:, 0:4].bitcast(mybir.dt.bfloat16))
        b_half = singles.tile([64, 1], f32)
        nc.sync.dma_start(out=b_half, in_=b)
        nc.scalar.mul(out=b_half, in_=b_half, mul=0.5)

        # wt[:, k, :] = fp16(0.5 * W[:, 128k:128k+128].T)   ([128 d, 64 e])
        wt = singles.tile([128, KT, E], f16)
        for k in range(KT):
            # reuse the (not-yet-busy) transpose PSUM pool for weight prep
            wps = tp_psum.tile([128, 2, 512], f32, tag="tp")
            nc.tensor.matmul(
                wps[:, 0, :E],
                w_nat[:, k * 128 : (k + 1) * 128],
                ident[:64, :64],
                is_transpose=True,
            )
            nc.scalar.mul(out=wt[:, k, :], in_=wps[:, 0, :E], mul=0.5)

        # s[p, g, e] = 0.5 * (x @ W.T + b)[token p*32+g, e]
        s_sb = singles.tile([128, GROUPS, E], f32)

        pending_outs = []

        def flush_outs():
            while pending_outs:
                g0, g1, pn = pending_outs.pop(0)
                nc.scalar.dma_start(out=out_v[:, g0:g1, :], in_=pn)

        # ---- entmax unit solver ------------------------------------------
        def emit_unit(g0, g1):
            G = g1 - g0
            sv = s_sb[:, g0:g1, :]

            def sm(tag):
                return sm_pool.tile([128, G], f32, name=f"{tag}{g0}", tag=f"{tag}{g0}")

            mx = sm("mx")
            nc.vector.tensor_reduce(out=mx, in_=sv, axis=AX.X, op=OP.max)
            tau = sm("tau")
            nc.vector.tensor_scalar_add(out=tau, in0=mx, scalar1=-1.0)
            taub = bcast(tau, E)

            d = big_pool.tile([128, G, E], f32, name=f"d{g0}", tag="d", bufs=2)
            r = big_pool.tile([128, G, E], f32, name=f"r{g0}", tag="r", bufs=2)
            q = d  # q = r*r overwrites d in place (d dead once r exists)
            fq, fr, inv, stp = sm("fq"), sm("fr"), sm("inv"), sm("stp")

            def feval():
                # all ops stay on DVE; scalar_tensor_tensor runs 2x_2p
                nc.vector.scalar_tensor_tensor(
                    out=d, in0=sv, scalar=1.0, in1=taub, op0=OP.mult, op1=OP.subtract
                )
                nc.vector.tensor_scalar_max(out=r, in0=d, scalar1=0.0)  # relu
                nc.vector.scalar_tensor_tensor(
                    out=q, in0=r, scalar=1.0, in1=r, op0=OP.mult, op1=OP.mult
                )
                nc.vector.tensor_reduce(out=fq, in_=q, axis=AX.X, op=OP.add)

            for _ in range(N_EVALS - 1):
                feval()
                nc.vector.tensor_reduce(out=fr, in_=r, axis=AX.X, op=OP.add)
                # tau += (fq - 1) * 0.5 / fr
                nc.vector.reciprocal(out=inv, in_=fr)
                nc.vector.scalar_tensor_tensor(
                    out=stp, in0=fq, scalar=-1.0, in1=inv, op0=OP.add, op1=OP.mult
                )
                nc.vector.scalar_tensor_tensor(
                    out=tau, in0=stp, scalar=0.5, in1=tau, op0=OP.mult, op1=OP.add
                )

            # final: p = q / sum(q)
            feval()
            rcp = sm("rcp")
            nc.vector.reciprocal(out=rcp, in_=fq)
            pn = r  # final p overwrites r in place
            nc.vector.scalar_tensor_tensor(
                out=pn, in0=q, scalar=1.0, in1=bcast(rcp, E), op0=OP.mult, op1=OP.mult
            )
            # defer the out DMA: issuing it immediately would park the ACT
            # queue on the unit's completion sem and stall the next block's
            # copies; it is flushed at the next block's emission point.
            pending_outs.append((g0, g1, pn))

        # ---- streaming matmul phase --------------------------------------
        for bi, (gb0, gb1) in enumerate(BLOCKS):
            NG = gb1 - gb0
            F = NG * 128  # moving free size for this block
            pool = xin_pool if NG == 4 else xs_pool
            xin = pool.tile([128, NG, D], f32, tag=f"xin{NG}")
            nc.sync.dma_start(out=xin, in_=x_v[:, gb0:gb1, :])
            flush_outs()  # previous units' results are ready by now

            lg_ps_full = lg_psum.tile([64, 512], f32, tag="lgps")
            lg_ps = lg_ps_full[:, :F]
            KPT = 2  # k-tiles per PSUM tile / copy
            for kg in range(KT // KPT):
                tp = tp_psum.tile([128, 2, 512], f32, tag="tp")
                tpf = tp.rearrange("p a (b f) -> p (a b) f", f=F)
                for i in range(KPT):
                    k = KPT * kg + i
                    for ch in range(NG):
                        # [128 tok, 128 d] -> [128 d, 128 tok] (fp32 2cyc)
                        nc.tensor.matmul(
                            tpf[:, i, ch * 128 : (ch + 1) * 128],
                            xin[:, ch, k * 128 : (k + 1) * 128],
                            ident,
                            is_transpose=True,
                            skip_group_check=True,
                        )
                xt = xt_pool.tile([128, (2 * 512) // F, F], f16, tag="xt")
                # all PSUM->SBUF copies on ACT: the DVE queue must never
                # carry stream work, or units cascade into the next block
                nc.scalar.copy(out=xt, in_=tpf)
                for i in range(KPT):
                    k = KPT * kg + i
                    nc.tensor.matmul(
                        lg_ps,
                        wt[:, k, :],
                        xt[:, i, :],
                        start=(k == 0),
                        stop=(k == KT - 1),
                    )
            # epilogue: add 0.5*b (per-partition = per-expert here)
            lg_sb_full = lg_pool.tile([64, 512], f32, tag="lgsb")
            lg_sb = lg_sb_full[:, :F]
            nc.scalar.activation(
                out=lg_sb, in_=lg_ps, func=AF.Identity, bias=b_half, scale=1.0
            )
            nc.tensor.ldweights(lg_sb[:, 0:4].bitcast(mybir.dt.bfloat16))
            # de-transpose [64, F] -> [128, NG, 64] in one PSUM tile
            s_ps_full = s_psum.tile([128, 4, E], f32, tag="sps")
            s_ps = s_ps_full[:, :NG, :]
            for ch in range(NG):
                nc.tensor.matmul(
                    s_ps[:, ch, :],
                    lg_sb[:, ch * 128 : (ch + 1) * 128],
                    ident[:64, :64],
                    is_transpose=True,
                    skip_group_check=True,
                )
            nc.vector.tensor_copy(out=s_sb[:, gb0:gb1, :], in_=s_ps)

            for g0, g1 in UNITS_AFTER_BLOCK.get(bi, []):
                emit_unit(g0, g1)
        flush_outs()

    _legalize_waits(nc)

    _BUILT = nc
    return nc


def _legalize_waits(nc):
    # Walrus codegen rejects instructions whose ISA struct lacks slots for
    # all the sync waits Tile attached (most structs fit only one). Legalize:
    # cap every instruction at one wait and hoist the extras onto same-engine
    # carrier InstDrains placed just before (drains carry sync_info in Tile's
    # own barriers, ~12ns each).
    from concourse import mybir

    ndrain = 0
    for fn in nc.m.functions:
        for blk in fn.blocks:
            new_insts = []
            for inst in blk.instructions:
                si = inst.sync_info
                if si is not None and si.on_wait and len(si.on_wait) > 1:
                    for w in list(si.on_wait)[:-1]:
                        d = mybir.InstDrain(
                            name=f"{inst.name}-wsplit{ndrain}",
                            ins=[],
                            outs=[],
                            bass_is_fusable=False,
                        )
                        ndrain += 1
                        d.engine = inst.engine
                        d.sync_info = mybir.SyncInfo(on_wait=[w], on_update=[])
                        new_insts.append(d)
                    inst.sync_info = mybir.SyncInfo(
                        on_wait=[si.on_wait[-1]], on_update=si.on_update
                    )
                new_insts.append(inst)
            blk.instructions = new_insts


def _run(x, W, b, trace=False):
    from concourse.bass_utils import run_bass_kernel_spmd

    nc = _build()
    x = np.ascontiguousarray(x, dtype=np.float32)
    W = np.ascontiguousarray(W, dtype=np.float32)
    b2 = np.ascontiguousarray(np.asarray(b, dtype=np.float32).reshape(E, 1))
    in_maps = [
        {
            "x": x[c * TOK_PER_CORE : (c + 1) * TOK_PER_CORE],
            "W": W,
            "b": b2,
        }
        for c in range(N_CORES)
    ]
    res = run_bass_kernel_spmd(nc, in_maps, core_ids=list(range(N_CORES)), trace=trace)
    full = np.concatenate([r["out"] for r in res.results], axis=0)
    return full, res


def kernel(x, W, b):
    full, _ = _run(x, W, b, trace=False)
    return full


# revision 3
# speedup vs baseline: 6.3118x; 6.3118x over previous
"""AlphaEntmaxRouter (alpha=1.5) Trainium2 kernel, v2.

Full inputs -> full output. Data-parallel over 8 NeuronCores (token dim
sharded 4096/core), weights replicated.

Per core:
  - x [4096, 2048] fp32 streamed via plain HWDGE DMAs on the SP queue (full
    rate; no SWDGE cast): six 512-token blocks, then four 256-token
    sub-blocks so the post-stream pipeline tail is short.
  - PE transposes each fp32 x chunk in transpose mode (fp32, 2 cyc/row)
    into PSUM; the ACT PSUM->SBUF copies downcast to fp16 in flight.
  - W-stationary fp16 matmul (1 cyc/row) accumulates logits^T
    [64, NG*128] in fp32 PSUM over the 16 k-tiles; bias + the 0.5 entmax
    scale are folded into the transposed weights / epilogue; PE
    re-transposes logits into s = 0.5*(x@W.T+b) laid out
    [128 part, 32 group, 64 expert].
  - entmax-1.5 tau from 4 Newton updates + final eval (5 evals) on
    f(tau) = sum_e relu(s_e - tau)^2 - 1 starting at tau0 = max(s)-1
    (monotone convergence from below; ~2e-3 rel err on this data vs the
    reference's 25-step bisection, well under the 2e-2 gate).
    Every eval op stays on DVE (fused scalar_tensor_tensor forms at the
    2x "2x_2p" rate where eligible) so each unit is a back-to-back
    throughput-bound chain with no cross-engine stalls.  Units are emitted
    interleaved into the stream right after their source block so the
    engine's in-order queue overlaps solver work with later blocks.
  - p = relu(s-tau)^2 normalized by its sum, DMA'd out per unit on the ACT
    HWDGE queue, deferred to the next block's emission point so the queue
    never parks on an unfinished unit.
  - A post-schedule pass (_legalize_waits) splits multi-wait instructions
    for this walrus build.
"""

import numpy as np

N_TOKENS = 32768
D = 2048
E = 64
N_CORES = 8
TOK_PER_CORE = N_TOKENS // N_CORES  # 4096
KT = D // 128  # 16 k-tiles
N_EVALS = 5  # 4 Newton updates + final eval

_BUILT = None


def _build():
    global _BUILT
    if _BUILT is not None:
        return _BUILT

    from contextlib import ExitStack

    import concourse.bass as bass
    import concourse.tile as tile
    from concourse import mybir
    from concourse.masks import make_identity

    f32 = mybir.dt.float32
    OP = mybir.AluOpType
    AF = mybir.ActivationFunctionType
    AX = mybir.AxisListType

    GROUPS = TOK_PER_CORE // 128  # 32
    # blocks as group ranges: 6 fat blocks then 4 small ones for a short tail
    BLOCKS = [(4 * i, 4 * i + 4) for i in range(8)]
    # every block is followed by its own solver unit (1:1), so solver work
    # trails the stream by exactly one block pipeline
    UNITS_AFTER_BLOCK = {i: [rng] for i, rng in enumerate(BLOCKS)}

    nc = bass.Bass("TRN2", debug=False)
    x = nc.dram_tensor("x", [TOK_PER_CORE, D], f32, kind="ExternalInput").ap()
    W = nc.dram_tensor("W", [E, D], f32, kind="ExternalInput").ap()
    b = nc.dram_tensor("b", [E, 1], f32, kind="ExternalInput").ap()
    out = nc.dram_tensor("out", [TOK_PER_CORE, E], f32, kind="ExternalOutput").ap()

    # token t = p*32 + g
    x_v = x.rearrange("(p g) d -> p g d", p=128)
    out_v = out.rearrange("(p g) e -> p g e", p=128)

    def bcast(ap2d, n):
        """[P, G] AP -> [P, G, n] stride-0 broadcast AP."""
        return bass.AP(tensor=ap2d.tensor, offset=ap2d.offset, ap=[*ap2d.ap, [0, n]])

    with tile.TileContext(nc) as tc, ExitStack() as ctx:
        singles = ctx.enter_context(tc.tile_pool(name="singles", bufs=1))
        xin_pool = ctx.enter_context(tc.tile_pool(name="xin", bufs=3))
        xs_pool = ctx.enter_context(tc.tile_pool(name="xs", bufs=4))
        xt_pool = ctx.enter_context(tc.tile_pool(name="xt", bufs=3))
        lg_pool = ctx.enter_context(tc.tile_pool(name="lg", bufs=2))
        big_pool = ctx.enter_context(tc.tile_pool(name="big", bufs=2))
        sm_pool = ctx.enter_context(tc.tile_pool(name="sm", bufs=2))
        tp_psum = ctx.enter_context(tc.tile_pool(name="tp_ps", bufs=2, space="PSUM"))
        lg_psum = ctx.enter_context(tc.tile_pool(name="lg_ps", bufs=2, space="PSUM"))
        s_psum = ctx.enter_context(tc.tile_pool(name="s_ps", bufs=2, space="PSUM"))

        # ---- constants / weight prep -------------------------------------
        f16 = mybir.dt.float16
        ident = singles.tile([128, 128], f32)
        make_identity(nc, ident)

        # first x block's DMA goes out first: it paces the whole stream,
        # while weight prep only needs to finish before the first matmul
        xin0 = xin_pool.tile([128, 4, D], f32, tag="xin4")
        nc.sync.dma_start(out=xin0, in_=x_v[:, 0:4, :])

        w_nat = singles.tile([64, D], f32)
        nc.scalar.dma_start(out=w_nat, in_=W)
        # Wait-absorber: transpose-mode matmul's LDW struct only fits one
        # sync wait; soak the DMA-completion wait with a tiny standalone
        # bf16 ldweights so later PE instructions inherit it in order.
        nc.tensor.ldweights(w_nat[:, 0:4].bitcast(mybir.dt.bfloat16))
        b_half = singles.tile([64, 1], f32)
        nc.scalar.dma_start(out=b_half, in_=b)
        nc.scalar.mul(out=b_half, in_=b_half, mul=0.5)

        # wt[:, k, :] = fp16(0.5 * W[:, 128k:128k+128].T)   ([128 d, 64 e])
        wt = singles.tile([128, KT, E], f16)
        for k in range(KT):
            # reuse the (not-yet-busy) transpose PSUM pool for weight prep
            wps = tp_psum.tile([128, 2, 512], f32, tag="tp")
            nc.tensor.matmul(
                wps[:, 0, :E],
                w_nat[:, k * 128 : (k + 1) * 128],
                ident[:64, :64],
                is_transpose=True,
            )
            nc.scalar.mul(out=wt[:, k, :], in_=wps[:, 0, :E], mul=0.5)

        # s[p, g, e] = 0.5 * (x @ W.T + b)[token p*32+g, e]
        s_sb = singles.tile([128, GROUPS, E], f32)

        pending_outs = []

        def flush_outs():
            while pending_outs:
                g0, g1, pn = pending_outs.pop(0)
                nc.scalar.dma_start(out=out_v[:, g0:g1, :], in_=pn)

        # ---- entmax unit solver ------------------------------------------
        def emit_unit_pair(ga, gb):
            """Two half-size units with op-emission zipped so their serial
            chains interleave on the DVE queue (for the post-stream tail)."""
            gens = [emit_unit_steps(*ga), emit_unit_steps(*gb)]
            done = [False, False]
            while not all(done):
                for j, g in enumerate(gens):
                    if not done[j]:
                        try:
                            next(g)
                        except StopIteration:
                            done[j] = True

        def emit_unit(g0, g1):
            for _ in emit_unit_steps(g0, g1):
                pass

        def emit_unit_steps(g0, g1):
            G = g1 - g0
            sv = s_sb[:, g0:g1, :]

            def sm(tag):
                return sm_pool.tile([128, G], f32, name=f"{tag}{g0}", tag=f"{tag}{g0}")

            mx = sm("mx")
            nc.vector.tensor_reduce(out=mx, in_=sv, axis=AX.X, op=OP.max)
            yield
            tau = sm("tau")
            nc.vector.tensor_scalar_add(out=tau, in0=mx, scalar1=-1.0)
            taub = bcast(tau, E)
            yield

            d = big_pool.tile([128, G, E], f32, name=f"d{g0}", tag="d", bufs=2)
            r = big_pool.tile([128, G, E], f32, name=f"r{g0}", tag="r", bufs=2)
            q = d  # q = r*r overwrites d in place (d dead once r exists)
            fq, fr, inv, stp = sm("fq"), sm("fr"), sm("inv"), sm("stp")

            def feval():
                # all ops stay on DVE: back-to-back throughput, no hops
                nc.vector.scalar_tensor_tensor(
                    out=d, in0=sv, scalar=1.0, in1=taub, op0=OP.mult, op1=OP.subtract
                )
                yield
                nc.vector.tensor_scalar_max(out=r, in0=d, scalar1=0.0)  # relu
                yield
                nc.vector.scalar_tensor_tensor(
                    out=q, in0=r, scalar=1.0, in1=r, op0=OP.mult, op1=OP.mult
                )
                yield
                nc.vector.tensor_reduce(out=fq, in_=q, axis=AX.X, op=OP.add)
                yield

            for _ in range(N_EVALS - 1):
                yield from feval()
                nc.vector.tensor_reduce(out=fr, in_=r, axis=AX.X, op=OP.add)
                yield
                # tau += (fq - 1) * 0.5 / fr
                nc.vector.reciprocal(out=inv, in_=fr)
                nc.vector.scalar_tensor_tensor(
                    out=stp, in0=fq, scalar=-1.0, in1=inv, op0=OP.add, op1=OP.mult
                )
                nc.vector.scalar_tensor_tensor(
                    out=tau, in0=stp, scalar=0.5, in1=tau, op0=OP.mult, op1=OP.add
                )
                yield

            # final: p = q / sum(q)
            yield from feval()
            rcp = sm("rcp")
            nc.vector.reciprocal(out=rcp, in_=fq)
            pn = r  # final p overwrites r in place
            nc.vector.scalar_tensor_tensor(
                out=pn, in0=q, scalar=1.0, in1=bcast(rcp, E), op0=OP.mult, op1=OP.mult
            )
            # defer the out DMA: issuing it immediately would park the ACT
            # queue on the unit's completion sem and stall the next block's
            # copies; it is flushed at the next block's emission point.
            pending_outs.append((g0, g1, pn))

        # ---- streaming matmul phase --------------------------------------
        for bi, (gb0, gb1) in enumerate(BLOCKS):
            NG = gb1 - gb0
            F = NG * 128  # moving free size for this block
            if bi == 0:
                xin = xin0  # DMA already issued ahead of weight prep
            else:
                pool = xin_pool if NG == 4 else xs_pool
                xin = pool.tile([128, NG, D], f32, tag=f"xin{NG}")
                nc.sync.dma_start(out=xin, in_=x_v[:, gb0:gb1, :])
            flush_outs()  # previous units' results are ready by now

            lg_ps_full = lg_psum.tile([64, 512], f32, tag="lgps")
            lg_ps = lg_ps_full[:, :F]
            KPT = 2  # k-tiles per PSUM tile / copy
            for kg in range(KT // KPT):
                tp = tp_psum.tile([128, 2, 512], f32, tag="tp")
                tpf = tp.rearrange("p a (b f) -> p (a b) f", f=F)
                for i in range(KPT):
                    k = KPT * kg + i
                    for ch in range(NG):
                        # [128 tok, 128 d] -> [128 d, 128 tok] (fp32 2cyc)
                        nc.tensor.matmul(
                            tpf[:, i, ch * 128 : (ch + 1) * 128],
                            xin[:, ch, k * 128 : (k + 1) * 128],
                            ident,
                            is_transpose=True,
                            skip_group_check=True,
                        )
                xt = xt_pool.tile([128, (2 * 512) // F, F], f16, tag="xt")
                # all PSUM->SBUF copies on ACT: the DVE queue must never
                # carry stream work, or units cascade into the next block
                nc.scalar.copy(out=xt, in_=tpf)
                for i in range(KPT):
                    k = KPT * kg + i
                    nc.tensor.matmul(
                        lg_ps,
                        wt[:, k, :],
                        xt[:, i, :],
                        start=(k == 0),
                        stop=(k == KT - 1),
                    )
            # epilogue: add 0.5*b (per-partition = per-expert here)
            lg_sb_full = lg_pool.tile([64, 512], f32, tag="lgsb")
            lg_sb = lg_sb_full[:, :F]
            nc.scalar.activation(
                out=lg_sb, in_=lg_ps, func=AF.Identity, bias=b_half, scale=1.0
            )
            nc.tensor.ldweights(lg_sb[:, 0:4].bitcast(mybir.dt.bfloat16))
            # de-transpose [64, F] -> [128, NG, 64] in one PSUM tile
            s_ps_full = s_psum.tile([128, 4, E], f32, tag="sps")
            s_ps = s_ps_full[:, :NG, :]
            for ch in range(NG):
                nc.tensor.matmul(
                    s_ps[:, ch, :],
                    lg_sb[:, ch * 128 : (ch + 1) * 128],
                    ident[:64, :64],
                    is_transpose=True,
                    skip_group_check=True,
                )
            nc.vector.tensor_copy(out=s_sb[:, gb0:gb1, :], in_=s_ps)

            for g0, g1 in UNITS_AFTER_BLOCK.get(bi, []):
                emit_unit(g0, g1)
        flush_outs()

    _legalize_waits(nc)

    _BUILT = nc
    return nc


def _legalize_waits(nc):
    # Walrus codegen rejects instructions whose ISA struct lacks slots for
    # all the sync waits Tile attached (most structs fit only one). Legalize:
    # cap every instruction at one wait and hoist the extras onto same-engine
    # carrier InstDrains placed just before (drains carry sync_info in Tile's
    # own barriers, ~12ns each).
    from concourse import mybir

    ndrain = 0
    for fn in nc.m.functions:
        for blk in fn.blocks:
            new_insts = []
            for inst in blk.instructions:
                si = inst.sync_info
                if si is not None and si.on_wait and len(si.on_wait) > 1:
                    for w in list(si.on_wait)[:-1]:
                        d = mybir.InstDrain(
                            name=f"{inst.name}-wsplit{ndrain}",
                            ins=[],
                            outs=[],
                            bass_is_fusable=False,
                        )
                        ndrain += 1
                        d.engine = inst.engine
                        d.sync_info = mybir.SyncInfo(on_wait=[w], on_update=[])
                        new_insts.append(d)
                    inst.sync_info = mybir.SyncInfo(
                        on_wait=[si.on_wait[-1]], on_update=si.on_update
                    )
                new_insts.append(inst)
            blk.instructions = new_insts


def _run(x, W, b, trace=False):
    from concourse.bass_utils import run_bass_kernel_spmd

    nc = _build()
    x = np.ascontiguousarray(x, dtype=np.float32)
    W = np.ascontiguousarray(W, dtype=np.float32)
    b2 = np.ascontiguousarray(np.asarray(b, dtype=np.float32).reshape(E, 1))
    in_maps = [
        {
            "x": x[c * TOK_PER_CORE : (c + 1) * TOK_PER_CORE],
            "W": W,
            "b": b2,
        }
        for c in range(N_CORES)
    ]
    res = run_bass_kernel_spmd(nc, in_maps, core_ids=list(range(N_CORES)), trace=trace)
    full = np.concatenate([r["out"] for r in res.results], axis=0)
    return full, res


def kernel(x, W, b):
    full, _ = _run(x, W, b, trace=False)
    return full
